# revision 1
# baseline (speedup 1.0000x reference)
"""Trainium2 Bass kernel for nn_NodeModel (GNN message passing + 3-layer node MLP).

Strategy (node-parallel, 8 cores):
  - Host: sort edges by destination node, bucket them into 128-node tiles,
    pad each tile's edge list to K_CH chunks of 128 edges. Nodes are sharded
    contiguously across the 8 cores (12544 padded nodes each).
  - Device (per core, per 128-node tile):
      aggT[h, n] = sum_k edge_chunk_k[e, h].T @ onehot(col_local_k)[e, n]
      (one-hot built on DVE via iota==col compare; matmul accumulates in PSUM)
      then fused 3-layer MLP with LayerNorm + shifted-softplus, activations
      kept transposed [h, node]; LN stats computed after a PE transpose to
      [node, h]; scale/shift+softplus fused into one ACT op in [h, node].
  - -log(2) of ssp folded into the next layer's bias (host-precomputed);
    final layer subtracts it explicitly.
"""

import os
import sys

import numpy as np

sys.path.insert(0, "/opt/trn_rl_repo")

import bass_rust as _bass_rust
import ml_dtypes

from concourse import bacc, bass, hw_specs, mybir
from concourse import tile as tile_mod
from concourse.bass_utils import run_bass_kernel_spmd
from concourse.masks import make_identity


class _Bacc(bacc.Bacc):
    """Bacc with the ACT table chooser pinned to the single function set
    that holds Ln+Exp+Copy+Identity. The default greedy chooser alternates
    between per-func sets, costing a ~1.3us ACT_TABLE_LOAD per switch."""

    def insert_act_table_loads(self):
        has_activation = any(
            isinstance(i, mybir.InstActivation)
            for b in self.main_func.blocks
            for i in b.instructions
        )
        if not has_activation:
            return
        keep = "natural_log_exp_and_others"
        tables = [
            (n, (s if n == keep else set()))
            for n, s in hw_specs.get_activation_tables(self.m.arch).items()
        ]
        _bass_rust.insert_act_table_loads(self, tables)


LOG2 = float(np.log(2.0))
N, E, H = 100000, 600000, 128
NC = 8
P = 128
TPC = 98                 # 128-node tiles per core
NPC = TPC * P            # nodes per core (12544)
NPAD = NPC * NC          # padded node count (100352)
NT = NPAD // P           # total node tiles (784)

F32 = mybir.dt.float32
F32R = mybir.dt.float32r
BF16 = mybir.dt.bfloat16

LAST_RESULT = None  # BassKernelResults of the most recent run (for profiling)


def _host_prep(x, edge_index, edge_attr):
    col = np.asarray(edge_index)[1].astype(np.int64)
    ea = np.ascontiguousarray(np.asarray(edge_attr, dtype=np.float32))
    order = np.argsort(col, kind="stable")
    col_s = col[order]
    tile_of = col_s >> 7
    counts = np.bincount(tile_of, minlength=NT)
    K = int(np.ceil(counts.max() / P))
    S = K * P
    starts = np.zeros(NT + 1, np.int64)
    starts[1:] = np.cumsum(counts)
    pos = np.arange(E) - starts[tile_of]
    slot = tile_of * S + pos
    slot_edge = np.zeros(NT * S, np.int64)
    slot_edge[slot] = order
    col_local = np.full(NT * S, 128.0, np.float32)
    col_local[slot] = (col_s & 127).astype(np.float32)
    payload = ea[slot_edge]  # [NT*S, H]

    x_pad = np.zeros((NPAD, H), np.float32)
    x_pad[:N] = np.asarray(x, dtype=np.float32)

    per_core = []
    for c in range(NC):
        r0, r1 = c * TPC * S, (c + 1) * TPC * S
        pay_c = np.ascontiguousarray(
            payload[r0:r1]
            .reshape(TPC, K, P, H)
            .transpose(0, 2, 1, 3)
            .reshape(TPC * P, K * H)
            .astype(ml_dtypes.bfloat16)
        )
        col_c = np.ascontiguousarray(
            col_local[r0:r1].reshape(TPC, K, P).transpose(2, 0, 1).reshape(P, TPC * K)
        )
        xt_c = np.ascontiguousarray(
            x_pad[c * NPC : (c + 1) * NPC]
            .reshape(TPC, P, H)
            .transpose(0, 2, 1)
            .reshape(TPC * P, P)
            .astype(ml_dtypes.bfloat16)
        )
        per_core.append((pay_c, col_c, xt_c))
    return K, per_core


def _build_program(K):
    # Bacc (not raw Bass): its compile pass splits multi-semaphore waits into
    # event-semaphore chains — walrus codegen allows only 1 wait per
    # instruction on this toolchain.
    nc = _Bacc("TRN2", target_bir_lowering=False, debug=False, num_devices=NC)

    edges_h = nc.dram_tensor("edges", [TPC * P, K * P], BF16, kind="ExternalInput")
    cols_h = nc.dram_tensor("cols", [P, TPC * K], F32, kind="ExternalInput")
    xt_h = nc.dram_tensor("xt", [TPC * P, P], BF16, kind="ExternalInput")
    w_h = {
        name: nc.dram_tensor(name, [P, P], BF16, kind="ExternalInput")
        for name in ("w1a", "w1b", "w2", "w3")
    }
    # b1,b2,b3,g1,g2,g3,be1,be2,be3 packed as columns of one tensor (one DMA,
    # one semaphore for every per-partition vector operand).
    vecs_h = nc.dram_tensor("vecs", [P, 9], F32, kind="ExternalInput")
    iota_h = nc.dram_tensor("iota", [P, P], F32, kind="ExternalInput")
    out_h = nc.dram_tensor("out", [TPC * P, P], F32, kind="ExternalOutput")
    VIDX = {n: i for i, n in enumerate(("b1", "b2", "b3", "g1", "g2", "g3", "be1", "be2", "be3"))}

    with tile_mod.TileContext(nc) as tc:
        with (
            tc.tile_pool(name="const", bufs=1) as cpool,
            tc.tile_pool(name="edges", bufs=3) as epool,
            tc.tile_pool(name="xin", bufs=3) as xpool,
            tc.tile_pool(name="sel", bufs=4) as selpool,
            tc.tile_pool(name="work", bufs=3) as wpool,
            tc.tile_pool(name="stats", bufs=6) as spool,
            tc.tile_pool(name="psum", bufs=8, space="PSUM") as ppool,
        ):
            ident = cpool.tile([P, P], F32)
            make_identity(nc, ident[:])

            def transpose(dst_psum, src_sbuf):
                nc.tensor.transpose(dst_psum[:], src_sbuf[:], ident[:])
            iota = cpool.tile_from(iota_h[:])
            cols = cpool.tile_from(cols_h[:])
            W = {k: cpool.tile_from(h[:], name=f"w_{k}") for k, h in w_h.items()}
            vecs = cpool.tile_from(vecs_h[:])
            V = {n: vecs[:, i : i + 1] for n, i in VIDX.items()}
            eps = cpool.tile([P, 1], F32)
            nc.gpsimd.memset(eps[:], 1e-5)
            half = cpool.tile([P, 1], F32)
            nc.gpsimd.memset(half[:], 0.5)

            def layer(zT_psum, b, g, be, out_dtype=BF16):
                """zT_psum: [h_out, n] pre-activation in PSUM.
                Returns ssp(LN(zT + b) * g + be) as [h_out, n] in SBUF,
                including the -log2 shift (ln(0.5*exp(y) + 0.5))."""
                # NOTE: TensorScalar's ISA struct fits only ONE sync wait, so
                # everything here uses tensor_tensor with broadcast [P,1] APs.
                zbT = wpool.tile([P, P], F32, tag="zbT")
                nc.vector.tensor_tensor(
                    zbT[:], zT_psum[:], V[b].to_broadcast([P, P]),
                    op=mybir.AluOpType.add,
                )
                z_rm = ppool.tile([P, P], F32, tag="ps")
                transpose(z_rm, zbT)
                st6 = spool.tile([P, 6], F32, tag="st6")
                nc.vector.bn_stats(st6[:], z_rm[:])
                st2 = spool.tile([P, 2], F32, tag="st2")
                nc.vector.bn_aggr(st2[:], st6[:])
                # rsqrt(var + eps) = exp(-0.5 * ln(var + eps)); no ACT func
                # set holds both Sqrt and a softplus path, but Ln+Exp coexist.
                lnv = spool.tile([P, 1], F32, tag="lnv")
                nc.scalar.activation(
                    lnv[:], st2[:, 1:2], mybir.ActivationFunctionType.Ln,
                    bias=eps[:, 0:1],
                )
                rsig = spool.tile([P, 1], F32, tag="rsig")
                nc.scalar.activation(
                    rsig[:], lnv[:], mybir.ActivationFunctionType.Exp, scale=-0.5
                )
                zc = wpool.tile([P, P], F32, tag="zc")
                nc.vector.tensor_tensor(
                    zc[:], z_rm[:], st2[:, 0:1].to_broadcast([P, P]),
                    op=mybir.AluOpType.subtract,
                )
                zn = wpool.tile([P, P], F32, tag="zn")
                zn_eng = nc.gpsimd if os.environ.get("KERNEL_ZN_GPS", "1") == "1" else nc.vector
                zn_eng.tensor_tensor(
                    zn[:], zc[:], rsig[:, 0:1].to_broadcast([P, P]),
                    op=mybir.AluOpType.mult,
                )
                znT = ppool.tile([P, P], F32, tag="ps")
                transpose(znT, zn)
                # ssp(y) = softplus(y) - log2 = ln(0.5*exp(y) + 0.5), with
                # y = g*zn + be. LN output is bounded (|zn| <= sqrt(127)) so
                # exp cannot overflow.
                ez = wpool.tile([P, P], F32, tag="ez")
                nc.scalar.activation(
                    ez[:],
                    znT[:],
                    mybir.ActivationFunctionType.Exp,
                    bias=V[be],
                    scale=V[g],
                )
                spT = wpool.tile([P, P], out_dtype, tag="spT")
                nc.scalar.activation(
                    spT[:], ez[:], mybir.ActivationFunctionType.Ln,
                    bias=half[:, 0:1], scale=0.5,
                )
                return spT

            sel_eng = nc.gpsimd if os.environ.get("KERNEL_SEL_GPS", "0") == "1" else nc.vector
            n_tiles = int(os.environ.get("KERNEL_TPC", str(TPC)))
            for t in range(n_tiles):
                ed = epool.tile([P, K * P], BF16, tag="ed")
                nc.sync.dma_start(out=ed[:], in_=edges_h[t * P : (t + 1) * P, :])
                xt = xpool.tile([P, P], BF16, tag="xt")
                nc.sync.dma_start(out=xt[:], in_=xt_h[t * P : (t + 1) * P, :])

                aggT = ppool.tile([P, P], F32, tag="ps")
                for k in range(K):
                    sel = selpool.tile([P, P], BF16, tag="sel")
                    sel_eng.tensor_tensor(
                        sel[:],
                        cols[:, t * K + k : t * K + k + 1].to_broadcast([P, P]),
                        iota[:],
                        op=mybir.AluOpType.is_equal,
                    )
                    nc.tensor.matmul(
                        out=aggT[:],
                        lhsT=ed[:, k * P : (k + 1) * P],
                        rhs=sel[:],
                        start=(k == 0),
                        stop=(k == K - 1),
                    )
                aggS = wpool.tile([P, P], BF16, tag="aggS")
                nc.vector.tensor_copy(aggS[:], aggT[:])

                z1T = ppool.tile([P, P], F32, tag="ps")
                nc.tensor.matmul(out=z1T[:], lhsT=W["w1a"][:], rhs=xt[:], start=True, stop=False)
                nc.tensor.matmul(out=z1T[:], lhsT=W["w1b"][:], rhs=aggS[:], start=False, stop=True)
                h1T = layer(z1T, "b1", "g1", "be1")

                z2T = ppool.tile([P, P], F32, tag="ps")
                nc.tensor.matmul(out=z2T[:], lhsT=W["w2"][:], rhs=h1T[:], start=True, stop=True)
                h2T = layer(z2T, "b2", "g2", "be2")

                z3T = ppool.tile([P, P], F32, tag="ps")
                nc.tensor.matmul(out=z3T[:], lhsT=W["w3"][:], rhs=h2T[:], start=True, stop=True)
                h3T = layer(z3T, "b3", "g3", "be3", out_dtype=F32)
                nc.sync.dma_start(out=out_h[t * P : (t + 1) * P, :], in_=h3T[:])

    if not nc.is_finalized():
        nc.finalize()
    return nc


def kernel(
    x, edge_index, edge_attr,
    W1, b1, g1, be1, W2, b2, g2, be2, W3, b3, g3, be3,
):
    global LAST_RESULT
    W1 = np.asarray(W1, np.float32)
    W2 = np.asarray(W2, np.float32)
    W3 = np.asarray(W3, np.float32)

    K, per_core = _host_prep(x, edge_index, edge_attr)
    nc = _build_program(K)

    vecs = np.stack(
        [np.asarray(v, np.float32) for v in (b1, b2, b3, g1, g2, g3, be1, be2, be3)],
        axis=1,
    )  # [128, 9], column order must match VIDX in _build_program
    shared = {
        "w1a": np.ascontiguousarray(W1[:P]).astype(ml_dtypes.bfloat16),
        "w1b": np.ascontiguousarray(W1[P:]).astype(ml_dtypes.bfloat16),
        "w2": W2.astype(ml_dtypes.bfloat16),
        "w3": W3.astype(ml_dtypes.bfloat16),
        "vecs": np.ascontiguousarray(vecs),
        "iota": np.ascontiguousarray(
            np.broadcast_to(np.arange(P, dtype=np.float32), (P, P))
        ),
    }
    in_maps = [
        {"edges": pay_c, "cols": col_c, "xt": xt_c, **shared}
        for (pay_c, col_c, xt_c) in per_core
    ]

    trace = bool(int(os.environ.get("KERNEL_TRACE", "0")))
    res = run_bass_kernel_spmd(nc, in_maps, core_ids=list(range(NC)), trace=trace)
    LAST_RESULT = res

    out = np.concatenate(
        [
            r["out"].reshape(TPC, P, P).transpose(0, 2, 1).reshape(NPC, H)
            for r in res.results
        ],
        axis=0,
    )
    return np.ascontiguousarray(out[:N])



# revision 17
# speedup vs baseline: 1.9530x; 1.9530x over previous
"""Trainium2 Bass kernel for nn_NodeModel (GNN message passing + 3-layer node MLP).

v2 strategy (node-parallel, 8 cores, 512-node supertiles):
  - Host: sort edges by destination, bucket into 128-node tiles, pad each
    tile's edge list to K chunks of 128 edges. 100 tiles/core, grouped into
    25 supertiles of 4 tiles (512 nodes).
  - Device per supertile:
      agg:   one-hot via DVE/GPSIMD is_equal built [128,512]-wide (4 chunks
             per op via strided/broadcast APs), matmul-accumulated per tile.
      MLP:   z computed col-major [h, n] with W-stationary 512-wide matmuls.
             LayerNorm stats on the PE: per-chunk matmuls with zb/zsq as the
             stationary operand against +-ones/H vectors give -mu and E[z^2]
             as [node,1] PSUM columns; small-ops run on [128,4] tiles.
             Per-node normalize is fused into a per-chunk ACT Exp
             (scale=rstd).  gamma is folded into the forward transpose as a
             diag(g) rhs; the -mu*rstd x g term is added by a rank-1 matmul;
             beta is folded into the final Ln via per-partition EB=0.5*e^be
             scale: act_next = ln(EB * exp(g*(z-mu)*rstd) + 0.5) == ssp out.
  - Output returned bf16 from device, cast to f32 on host.
"""

import os
import sys

import numpy as np

sys.path.insert(0, "/opt/trn_rl_repo")

import bass_rust as _bass_rust
import ml_dtypes

from concourse import bacc, bass, hw_specs, mybir
from concourse import tile as tile_mod
from concourse.bass_utils import run_bass_kernel_spmd
from concourse.masks import make_identity


class _Bacc(bacc.Bacc):
    """Bacc with the ACT table chooser pinned to natural_log_exp_and_others
    (holds Ln+Exp+Identity+Copy+Square), avoiding ~1.3us table swaps."""

    def insert_act_table_loads(self):
        has_activation = any(
            isinstance(i, mybir.InstActivation)
            for b in self.main_func.blocks
            for i in b.instructions
        )
        if not has_activation:
            return
        keep = "natural_log_exp_and_others"
        tables = [
            (n, (s if n == keep else set()))
            for n, s in hw_specs.get_activation_tables(self.m.arch).items()
        ]
        _bass_rust.insert_act_table_loads(self, tables)


N, E, H = 100000, 600000, 128
NC = 8
P = 128
TPC = 100                # 128-node tiles per core
ST = 4                   # tiles per supertile
NST = TPC // ST          # supertiles per core (25)
SW = ST * P              # supertile width in nodes (512)
NPC = TPC * P            # nodes per core (12800)
NPAD = NPC * NC          # padded node count (102400)
NT = NPAD // P           # total node tiles (800)

F32 = mybir.dt.float32
BF16 = mybir.dt.bfloat16

LAST_RESULT = None  # BassKernelResults of the most recent run (for profiling)


def _host_prep(x, edge_index, edge_attr):
    col = np.asarray(edge_index)[1].astype(np.int64)
    ea = np.ascontiguousarray(np.asarray(edge_attr, dtype=np.float32))
    order = np.argsort(col, kind="stable")
    col_s = col[order]
    tile_of = col_s >> 7
    counts = np.bincount(tile_of, minlength=NT)
    K = int(np.ceil(counts.max() / P))
    S = K * P
    starts = np.zeros(NT + 1, np.int64)
    starts[1:] = np.cumsum(counts)
    pos = np.arange(E) - starts[tile_of]
    slot = tile_of * S + pos
    slot_edge = np.zeros(NT * S, np.int64)
    slot_edge[slot] = order
    col_local = np.full(NT * S, 128.0, np.float32)
    col_local[slot] = (col_s & 127).astype(np.float32)
    payload = ea[slot_edge]  # [NT*S, H]

    x_pad = np.zeros((NPAD, H), np.float32)
    x_pad[:N] = np.asarray(x, dtype=np.float32)

    per_core = []
    for c in range(NC):
        r0, r1 = c * TPC * S, (c + 1) * TPC * S
        # edges: [NST*P, ST*K*P] bf16; row = st*128+e, col = (t*K+k)*128+h
        ed_c = np.ascontiguousarray(
            payload[r0:r1]
            .reshape(NST, ST, K, P, H)
            .transpose(0, 3, 1, 2, 4)
            .reshape(NST * P, ST * K * H)
            .astype(ml_dtypes.bfloat16)
        )
        # cols: [P, NST*ST*K] bf16; col index = st*ST*K + t*K + k
        col_c = np.ascontiguousarray(
            col_local[r0:r1]
            .reshape(NST, ST, K, P)
            .transpose(3, 0, 1, 2)
            .reshape(P, NST * ST * K)
            .astype(ml_dtypes.bfloat16)
        )
        # xt: [NST*P, SW] bf16 col-major per supertile; row st*128+h, col t*128+n
        xt_c = np.ascontiguousarray(
            x_pad[c * NPC : (c + 1) * NPC]
            .reshape(NST, ST, P, H)
            .transpose(0, 3, 1, 2)
            .reshape(NST * P, SW)
            .astype(ml_dtypes.bfloat16)
        )
        per_core.append((ed_c, col_c, xt_c))
    return K, per_core


def _build_program(K):
    nc = _Bacc("TRN2", target_bir_lowering=False, debug=False, num_devices=NC)

    ed_h = nc.dram_tensor("edges", [NST * P, ST * K * P], BF16, kind="ExternalInput")
    cols_h = nc.dram_tensor("cols", [P, NST * ST * K], BF16, kind="ExternalInput")
    xt_h = nc.dram_tensor("xt", [NST * P, SW], BF16, kind="ExternalInput")
    w_h = {
        name: nc.dram_tensor(name, [P, P], BF16, kind="ExternalInput")
        for name in ("w1a", "w1b", "w2", "w3")
    }
    diag_h = {
        l: nc.dram_tensor(f"diag{l}", [P, P], BF16, kind="ExternalInput")
        for l in range(3)
    }
    iota_h = nc.dram_tensor("iota4", [P, SW], BF16, kind="ExternalInput")
    # g_l replicated across all partitions (rank-1 rhs needs matching base
    # partition at 32-aligned offsets)
    grows_h = nc.dram_tensor("grows", [3 * P, P], BF16, kind="ExternalInput")
    # vecs columns: b1,b2,b3, EB1,EB2,EB3, eps
    vecs_h = nc.dram_tensor("vecs", [P, 7], F32, kind="ExternalInput")
    # onesh columns (bf16): [-1/H, +1/H]
    onesh_h = nc.dram_tensor("onesh", [P, 2], BF16, kind="ExternalInput")
    out_h = nc.dram_tensor("out", [NST * P, SW], BF16, kind="ExternalOutput")

    sel_gps = int(os.environ.get("KERNEL_SEL_GPS", "0"))  # gpsimd lacks is_equal
    aggs_eng = os.environ.get("KERNEL_AGGS_ENG", "vector")
    mrs_eng = os.environ.get("KERNEL_MRS_ENG", "vector")
    zsq_gps = os.environ.get("KERNEL_ZSQ_GPS", "1") == "1"
    n_st = int(os.environ.get("KERNEL_NST", str(NST)))

    with tile_mod.TileContext(nc) as tc:
        with (
            tc.tile_pool(name="const", bufs=1) as cpool,
            tc.tile_pool(name="ed", bufs=2) as epool,
            tc.tile_pool(name="xin", bufs=3) as xpool,
            tc.tile_pool(name="sel", bufs=4) as selpool,
            tc.tile_pool(name="work", bufs=3) as wpool,
            tc.tile_pool(name="small", bufs=3) as spool,
            tc.tile_pool(name="pagg", bufs=2, space="PSUM") as pagg,
            tc.tile_pool(name="pz", bufs=2, space="PSUM") as pz,
            tc.tile_pool(name="pzrm", bufs=2, space="PSUM") as pzrm,
            tc.tile_pool(name="pmisc", bufs=2, space="PSUM") as pmisc,
        ):
            identB = cpool.tile([P, P], BF16)
            make_identity(nc, identB[:])
            iota4 = cpool.tile_from(iota_h[:])
            colst = cpool.tile_from(cols_h[:])
            cols4 = colst.rearrange("p (s t k) -> p s t k", s=NST, t=ST, k=K)
            W = {k: cpool.tile_from(h[:], name=f"w_{k}") for k, h in w_h.items()}
            DG = {l: cpool.tile_from(h[:], name=f"dg_{l}") for l, h in diag_h.items()}
            GR = {
                l: cpool.tile_from(grows_h[l * P : (l + 1) * P, :], name=f"gr_{l}")
                for l in range(3)
            }
            vecs = cpool.tile_from(vecs_h[:])
            onesh = cpool.tile_from(onesh_h[:])
            B = {l: vecs[:, l : l + 1] for l in range(3)}
            EB = {l: vecs[:, 3 + l : 4 + l] for l in range(3)}
            epsap = vecs[:, 6:7]
            half = cpool.tile([P, 1], F32)
            nc.gpsimd.memset(half[:], 0.5)

            def sel_engine(k):
                return nc.gpsimd if (k % K) < sel_gps else nc.vector

            def drain_eng(name):
                return {"act": None, "vector": nc.vector, "gpsimd": nc.gpsimd}[name]

            def layer(st, l, z_ps, out_dtype=BF16):
                """z_ps: [h, SW] pre-activation (no bias) in PSUM, col-major.
                Returns act = ln(EB*exp(g*LN(z+b)) + 0.5) as [h, SW] bf16."""
                zb = wpool.tile([P, SW], BF16, tag="zb")
                nc.scalar.activation(
                    zb[:], z_ps[:], mybir.ActivationFunctionType.Identity,
                    bias=B[l],
                )
                zsq = wpool.tile([P, SW], BF16, tag="zsq")
                zsq_eng = nc.gpsimd if zsq_gps else nc.vector
                zsq_eng.tensor_tensor(
                    zsq[:], zb[:], zb[:], op=mybir.AluOpType.mult
                )
                z_rm = pzrm.tile([P, SW], F32, tag="zrm")
                misc = pmisc.tile([P, SW], F32, tag="misc")
                stats = misc[:, 0:8]      # cols 0:4 = -mu, 4:8 = E[z^2]
                for c in range(ST):
                    cs = slice(c * P, (c + 1) * P)
                    # -mu column
                    nc.tensor.matmul(
                        out=stats[:, c : c + 1], lhsT=zb[:, cs],
                        rhs=onesh[:, 0:1], start=True, stop=True,
                    )
                    # E[z^2] column
                    nc.tensor.matmul(
                        out=stats[:, 4 + c : 5 + c], lhsT=zsq[:, cs],
                        rhs=onesh[:, 1:2], start=True, stop=True,
                    )
                musq = spool.tile([P, 4], F32, tag="musq")
                nc.scalar.activation(
                    musq[:], stats[:, 0:4], mybir.ActivationFunctionType.Square
                )
                var = spool.tile([P, 4], F32, tag="var")
                nc.vector.tensor_tensor(
                    var[:], stats[:, 4:8], musq[:], op=mybir.AluOpType.subtract
                )
                lnv = spool.tile([P, 4], F32, tag="lnv")
                nc.scalar.activation(
                    lnv[:], var[:], mybir.ActivationFunctionType.Ln, bias=epsap
                )
                rstd = spool.tile([P, 4], F32, tag="rstd")
                nc.scalar.activation(
                    rstd[:], lnv[:], mybir.ActivationFunctionType.Exp, scale=-0.5
                )
                # -mu rows: transpose each [128,1] column of -mu onto 32-aligned
                # partitions {0,32,64,96} (matmul base-partition constraint)
                mrs = spool.tile([P, 4], BF16, tag="mrs")
                nc.vector.tensor_copy(mrs[:], stats[:, 0:4])
                # row c lands at partition (c%2)*32, free range 256+(c//2)*128
                for c in range(ST):
                    pb = (c % 2) * 32
                    fb = 256 + (c // 2) * P
                    nc.tensor.matmul(
                        out=misc[pb : pb + 1, fb : fb + P],
                        lhsT=mrs[:, c : c + 1], rhs=identB[:],
                        start=True, stop=True,
                    )
                mrsT = spool.tile([33, 2 * P], BF16, tag="mrsT")
                me = drain_eng(mrs_eng)
                if me is None:
                    nc.scalar.activation(
                        mrsT[:], misc[0:33, 256:512],
                        mybir.ActivationFunctionType.Copy,
                    )
                else:
                    me.tensor_copy(mrsT[:], misc[0:33, 256:512])
                e_rm = wpool.tile([P, SW], BF16, tag="erm")
                for c in range(ST):
                    pb = (c % 2) * 32
                    fb = (c // 2) * P
                    cs = slice(c * P, (c + 1) * P)
                    # forward transpose with gamma + rank-1 -mu*g, as one
                    # bank-contiguous accumulation group (a start=True matmul
                    # clears has_written for the whole PSUM bank, so no other
                    # group targeting this bank may open in between)
                    nc.tensor.matmul(
                        out=z_rm[:, cs], lhsT=zb[:, cs], rhs=DG[l][:],
                        start=True, stop=False,
                    )
                    nc.tensor.matmul(
                        out=z_rm[:, cs], lhsT=mrsT[pb : pb + 1, fb : fb + P],
                        rhs=GR[l][pb : pb + 1, :], start=False, stop=True,
                    )
                    # normalize + exp: e = exp((g*z - g*mu) * rstd) per-node
                    nc.scalar.activation(
                        e_rm[:, cs], z_rm[:, cs],
                        mybir.ActivationFunctionType.Exp,
                        scale=rstd[:, c : c + 1],
                    )
                e_ps = pz.tile([P, SW], F32, tag="z")
                for c in range(ST):
                    cs = slice(c * P, (c + 1) * P)
                    nc.tensor.matmul(
                        out=e_ps[:, cs], lhsT=e_rm[:, cs], rhs=identB[:],
                        start=True, stop=True,
                    )
                act = wpool.tile([P, SW], out_dtype, tag="act")
                nc.scalar.activation(
                    act[:], e_ps[:], mybir.ActivationFunctionType.Ln,
                    bias=half[:, 0:1], scale=EB[l],
                )
                return act

            for st in range(n_st):
                ed = epool.tile([P, ST * K * P], BF16, tag="ed")
                nc.sync.dma_start(out=ed[:], in_=ed_h[st * P : (st + 1) * P, :])
                xt = xpool.tile([P, SW], BF16, tag="xt")
                nc.sync.dma_start(out=xt[:], in_=xt_h[st * P : (st + 1) * P, :])

                aggP = pagg.tile([P, SW], F32, tag="agg")
                # build all K one-hot masks first, then issue matmuls t-major
                # so each tile's PSUM accumulation group is bank-contiguous
                sels = []
                for k in range(K):
                    sel = selpool.tile([P, SW], BF16, tag=f"sel{k}", bufs=2)
                    sel3 = sel.rearrange("p (t n) -> p t n", t=ST, n=P)
                    cin = cols4[:, st, :, k].unsqueeze(2).to_broadcast([P, ST, P])
                    iin = iota4.rearrange("p (t n) -> p t n", t=ST, n=P)
                    sel_engine(k).tensor_tensor(
                        sel3, cin, iin, op=mybir.AluOpType.is_equal
                    )
                    sels.append(sel3)
                for t in range(ST):
                    for k in range(K):
                        nc.tensor.matmul(
                            out=aggP[:, t * P : (t + 1) * P],
                            lhsT=ed[:, (t * K + k) * P : (t * K + k + 1) * P],
                            rhs=sels[k][:, t, :],
                            start=(k == 0), stop=(k == K - 1),
                        )
                aggS = wpool.tile([P, SW], BF16, tag="aggS")
                ae = drain_eng(aggs_eng)
                if ae is None:
                    nc.scalar.activation(
                        aggS[:], aggP[:], mybir.ActivationFunctionType.Copy
                    )
                else:
                    ae.tensor_copy(aggS[:], aggP[:])

                z1 = pz.tile([P, SW], F32, tag="z")
                nc.tensor.matmul(out=z1[:], lhsT=W["w1a"][:], rhs=xt[:], start=True, stop=False)
                nc.tensor.matmul(out=z1[:], lhsT=W["w1b"][:], rhs=aggS[:], start=False, stop=True)
                a1 = layer(st, 0, z1)

                z2 = pz.tile([P, SW], F32, tag="z")
                nc.tensor.matmul(out=z2[:], lhsT=W["w2"][:], rhs=a1[:], start=True, stop=True)
                a2 = layer(st, 1, z2)

                z3 = pz.tile([P, SW], F32, tag="z")
                nc.tensor.matmul(out=z3[:], lhsT=W["w3"][:], rhs=a2[:], start=True, stop=True)
                a3 = layer(st, 2, z3, out_dtype=BF16)
                nc.sync.dma_start(out=out_h[st * P : (st + 1) * P, :], in_=a3[:])

    if not nc.is_finalized():
        nc.finalize()
    return nc


def kernel(
    x, edge_index, edge_attr,
    W1, b1, g1, be1, W2, b2, g2, be2, W3, b3, g3, be3,
):
    global LAST_RESULT
    W1 = np.asarray(W1, np.float32)
    W2 = np.asarray(W2, np.float32)
    W3 = np.asarray(W3, np.float32)

    K, per_core = _host_prep(x, edge_index, edge_attr)
    nc = _build_program(K)

    gs = [np.asarray(g, np.float32) for g in (g1, g2, g3)]
    bes = [np.asarray(b, np.float32) for b in (be1, be2, be3)]
    bs = [np.asarray(b, np.float32) for b in (b1, b2, b3)]
    vecs = np.stack(bs + [0.5 * np.exp(b) for b in bes] + [np.full(P, 1e-5, np.float32)], axis=1)
    grows = np.concatenate(
        [np.broadcast_to(g, (P, P)) for g in gs], axis=0
    )  # [3*P, P], g_l on every partition
    onesh = np.stack([np.full(P, -1.0 / H, np.float32), np.full(P, 1.0 / H, np.float32)], axis=1)
    shared = {
        "w1a": np.ascontiguousarray(W1[:P]).astype(ml_dtypes.bfloat16),
        "w1b": np.ascontiguousarray(W1[P:]).astype(ml_dtypes.bfloat16),
        "w2": W2.astype(ml_dtypes.bfloat16),
        "w3": W3.astype(ml_dtypes.bfloat16),
        "vecs": np.ascontiguousarray(vecs),
        "grows": grows.astype(ml_dtypes.bfloat16),
        "onesh": onesh.astype(ml_dtypes.bfloat16),
        "iota4": np.ascontiguousarray(
            np.tile(np.arange(P, dtype=np.float32), (P, ST))
        ).astype(ml_dtypes.bfloat16),
    }
    for l in range(3):
        shared[f"diag{l}"] = np.diag(gs[l]).astype(ml_dtypes.bfloat16)
    in_maps = [
        {"edges": ed_c, "cols": col_c, "xt": xt_c, **shared}
        for (ed_c, col_c, xt_c) in per_core
    ]

    trace = bool(int(os.environ.get("KERNEL_TRACE", "0")))
    res = run_bass_kernel_spmd(nc, in_maps, core_ids=list(range(NC)), trace=trace)
    LAST_RESULT = res

    out = np.concatenate(
        [
            np.asarray(r["out"], dtype=np.float32)
            .reshape(NST, P, ST, P)
            .transpose(0, 2, 3, 1)
            .reshape(NPC, H)
            for r in res.results
        ],
        axis=0,
    )
    return np.ascontiguousarray(out[:N])
